# revision 35
# baseline (speedup 1.0000x reference)
"""Multi-head attention block (B=2, S=2048, D=1024, H=16) on 8 trn2 cores.

Sharding: core c = (batch b = c//4, head-group g = c%4); each core computes
4 heads of one batch (Megatron column-shard of wq/wk/wv, row-shard of wo,
combined with data-parallel over batch). Host sums the 4 partial outputs
per batch and adds the (folded) bias.

v3 (pipelined bf16):
  - all matmul operands bf16 (halves DMA + fast weight load); PSUM fp32.
  - streaming structure per sb-block with the Q-projection of the next
    block interleaved before the last PV so the PE never waits on ACT;
    Tile overlaps engines across stages, PE stays HAM-warm.
  - causal structure per (ib, jb): only the visible i-suffix is computed
    by the S/exp/PV chain (runs); diagonal blocks get one 0/1 mask
    multiply on DVE; no zero-fill of masked regions (PV only contracts
    visible columns; per-column PSUM init via the has_written bit).
  - softmax row-sums via a ones-column appended to V (VS=65); softmax
    skips max-subtraction (scores are O(1)); r=1/l on DVE,
    partition-broadcast on GPSIMD.
  - PSUM budget (8 banks): scores tag "S" 2 x [128,1024]f32 slots (2
    banks each) shared with projection/out-proj accumulators; tag "O"
    4 x 1 bank for the PV accumulators.
"""

import numpy as np
import ml_dtypes

import concourse.bass as bass
import concourse.mybir as mybir
import concourse.tile as tile
from concourse import bacc
from concourse.bass_utils import run_bass_kernel_spmd

B, S, D, H = 2, 2048, 1024, 16
DK = D // H                  # 64
NCORES = 8
GROUPS = NCORES // B         # 4 head-groups
HPC = H // GROUPS            # 4 heads per core
OL = HPC * DK                # 256 local features
SB = 512                     # query (i) block
JB = 128                     # key (j) block
NSB = S // SB                # 4
NJB = S // JB                # 16
VS = DK + 1                  # V cols per head incl. ones column (65)
ND = D // 128                # 8 contraction chunks

F32 = mybir.dt.float32
BF16 = mybir.dt.bfloat16
BF = ml_dtypes.bfloat16

LAST_RUN = None  # stash of BassKernelResults for test harness inspection


def _classify_mask(mask2):
    """Per (ib, jb) schedule from the boolean mask [S, S] (True = visible).

    Returns (jlists, bias_tiles):
      jlists[ib] = list of (jb, runs, xs) for j-blocks with any visible
        entry; runs = [(k0, k1)] contiguous non-fully-masked i-subblock
        ranges (128 wide); xs = [(k, bias_idx)] partially-masked
        subblocks needing a 0/1 multiply.
      bias_tiles: deduped [JB, 128] f32 0/1 tiles (transposed: [j, i]).
    """
    jlists = []
    btiles = []
    bkeys = {}
    assert mask2.any(axis=1).all(), "mask has a fully-masked query row"
    for ib in range(NSB):
        jl = []
        for jb in range(NJB):
            sub = mask2[ib * SB:(ib + 1) * SB, jb * JB:(jb + 1) * JB]
            if not sub.any():
                continue
            runs = []
            xs = []
            start = None
            for k in range(SB // 128):
                s2 = sub[k * 128:(k + 1) * 128, :]
                if not s2.any():
                    if start is not None:
                        runs.append((start, k))
                        start = None
                    continue
                if start is None:
                    start = k
                if not s2.all():
                    t = np.where(s2, np.float32(1), np.float32(0)).T
                    key = t.tobytes()
                    if key not in bkeys:
                        bkeys[key] = len(btiles)
                        btiles.append(t)
                    xs.append((k, bkeys[key]))
            if start is not None:
                runs.append((start, SB // 128))
            jl.append((jb, runs, xs))
        # widest blocks first, narrowest last: the final block's exp (which
        # the last PV and the normalization wait on) is as short as possible.
        # Order is free: each PSUM column is initialized by its first writer
        # via the has_written bit (cleared once by the start=True matmul).
        jl.sort(key=lambda e: -sum(k1 - k0 for k0, k1 in e[1]))
        jlists.append(jl)
    return jlists, btiles


def _build(jlists, nbias, has_b):
    nc = bacc.Bacc()

    qTd = nc.dram_tensor("qT", [D, S], BF16, kind="ExternalInput")
    kTd = nc.dram_tensor("kT", [D, S], BF16, kind="ExternalInput")
    vTd = nc.dram_tensor("vT", [D, S], BF16, kind="ExternalInput")
    wqd = nc.dram_tensor("wqp", [128, ND * OL], BF16, kind="ExternalInput")
    wkd = nc.dram_tensor("wkp", [128, ND * OL], BF16, kind="ExternalInput")
    wvd = nc.dram_tensor("wvp", [128, ND * OL], BF16, kind="ExternalInput")
    wod = nc.dram_tensor("wop", [128, 2 * D], BF16, kind="ExternalInput")
    if has_b:
        bqd = nc.dram_tensor("bq", [OL, 1], F32, kind="ExternalInput")
        bkd = nc.dram_tensor("bk", [OL, 1], F32, kind="ExternalInput")
    if nbias:
        mbd = nc.dram_tensor("maskb", [nbias, JB, HPC * 128], BF16,
                             kind="ExternalInput")
    outd = nc.dram_tensor("out", [D, S], BF16, kind="ExternalOutput")

    with tile.TileContext(nc) as tc:
        with tc.tile_pool(name="consts", bufs=1) as consts, \
             tc.tile_pool(name="acts", bufs=48) as actp, \
             tc.tile_pool(name="pp", bufs=4) as pp, \
             tc.tile_pool(name="pnorm", bufs=2) as pnorm, \
             tc.tile_pool(name="pob", bufs=3) as pob, \
             tc.tile_pool(name="ps", bufs=2, space="PSUM") as psum:

            QT = [consts.tile([128, S], BF16, name=f"QT{t}") for t in range(2)]
            KT = [consts.tile([128, S], BF16, name=f"KT{t}") for t in range(2)]
            XT = [consts.tile([128, S], BF16, name=f"XT{t}") for t in range(2)]
            Vt = [consts.tile([128, HPC * VS], BF16, name=f"V{st}")
                  for st in range(S // 128)]
            wq_t = consts.tile([128, ND * OL], BF16, name="wqt")
            wk_t = consts.tile([128, ND * OL], BF16, name="wkt")
            wv_t = consts.tile([128, ND * OL], BF16, name="wvt")
            wo_t = consts.tile([128, 2 * D], BF16, name="wot")
            mb = [consts.tile([JB, HPC * 128], BF16, name=f"mb{i}")
                  for i in range(nbias)]
            dummy = consts.tile([1, 8], BF16, name="dummy")
            if has_b:
                bq_t = [consts.tile([128, 1], F32, name=f"bq{t}")
                        for t in range(2)]
                bk_t = [consts.tile([128, 1], F32, name=f"bk{t}")
                        for t in range(2)]

            # preload the exp table set during the initial DMA stall
            nc.vector.memset(dummy[:], 0.0)
            nc.scalar.activation(dummy[:], dummy[:],
                                 mybir.ActivationFunctionType.Exp)
            # warm the PE HAM clock-gate during the initial DMA stall:
            # ~3.4us of sustained matmuls lifts the PE from 1.2 to 2.4 GHz
            # before the first projection matmul issues (dummy data)
            wrm = consts.tile([128, SB], BF16, name="wrm")
            nc.vector.memset(wrm[:], 0.0)
            psw = psum.tile([128, SB], F32, tag="S", bufs=2, name="psw")
            for _ in range(22):
                nc.tensor.matmul(psw[:], wrm[:, 0:128], wrm[:],
                                 start=True, stop=True,
                                 skip_group_check=True)
            # ones columns of V (memset once; V evictions write cols 0:DK)
            for st in range(S // 128):
                v3 = Vt[st][:].rearrange("p (h c) -> p h c", c=VS)
                nc.vector.memset(v3[:, :, DK:VS], 1.0)

            act_tiles = {}

            def fetch_acts(tname, dram, pair):
                for d in range(ND):
                    at = actp.tile([128, 2 * SB], BF16, tag="act", bufs=48,
                                   name="at")
                    nc.sync.dma_start(
                        at[:],
                        dram[d * 128:(d + 1) * 128,
                             pair * 2 * SB:(pair + 1) * 2 * SB])
                    act_tiles[(tname, d, pair)] = at

            # setup DMAs in first-consumption order
            nc.sync.dma_start(wq_t[:], wqd[:, :])
            fetch_acts("q", qTd, 0)
            nc.sync.dma_start(wk_t[:], wkd[:, :])
            fetch_acts("k", kTd, 0)
            nc.sync.dma_start(wv_t[:], wvd[:, :])
            fetch_acts("v", vTd, 0)
            for i in range(nbias):
                nc.sync.dma_start(mb[i][:], mbd[i])
            nc.sync.dma_start(wo_t[:], wod[:, :])
            if has_b:
                for t in range(2):
                    nc.sync.dma_start(bq_t[t][:],
                                      bqd[t * 128:(t + 1) * 128, :])
                    nc.sync.dma_start(bk_t[t][:],
                                      bkd[t * 128:(t + 1) * 128, :])

            wq3 = wq_t[:].rearrange("p (d c) -> p d c", d=ND)
            wk3 = wk_t[:].rearrange("p (d c) -> p d c", d=ND)
            wv3 = wv_t[:].rearrange("p (d c) -> p d c", d=ND)
            wo3 = wo_t[:].rearrange("p (t c) -> p t c", t=2)

            def proj_qk(sb, dst, wview, src, bias):
                # psum[feat 128, i 512] = sum_d w[d, feat].T @ act[d, i]
                pair, half = sb // 2, sb % 2
                cols = slice(half * SB, (half + 1) * SB)
                ps = [psum.tile([128, SB], F32, tag="S", bufs=2, name="pjq")
                      for _ in range(2)]
                for d in range(ND):
                    at = act_tiles[(src, d, pair)]
                    for ot in range(2):
                        nc.tensor.matmul(
                            ps[ot][:],
                            wview[:, d, ot * 128:(ot + 1) * 128],
                            at[:, cols],
                            start=(d == 0), stop=(d == ND - 1))
                for ot in range(2):
                    dcols = dst[ot][:, sb * SB:(sb + 1) * SB]
                    if has_b:
                        nc.scalar.add(dcols, ps[ot][:], bias[ot][:])
                    else:
                        nc.scalar.copy(dcols, ps[ot][:])

            def proj_v(sb):
                # psum[s-subblock 128, feat 256] = act[d, s].T @ wv[d, :]
                pair, half = sb // 2, sb % 2
                for halfk in range(2):
                    psv = psum.tile([128, 2 * OL], F32, tag="S", bufs=2,
                                    name="pjv")
                    for k2 in range(2):
                        k = 2 * halfk + k2
                        for d in range(ND):
                            at = act_tiles[("v", d, pair)]
                            nc.tensor.matmul(
                                psv[:, k2 * OL:(k2 + 1) * OL],
                                at[:, half * SB + k * 128:
                                   half * SB + (k + 1) * 128],
                                wv3[:, d, :],
                                start=(d == 0), stop=(d == ND - 1),
                                skip_group_check=True)
                    for k2 in range(2):
                        st = sb * 4 + 2 * halfk + k2
                        v3 = Vt[st][:].rearrange("p (h c) -> p h c", c=VS)
                        nc.vector.tensor_copy(
                            v3[:, :, 0:DK],
                            psv[:, k2 * OL:(k2 + 1) * OL].rearrange(
                                "p (h c) -> p h c", c=DK))

            def attention_scores(ib, Ops):
                """jb loop: S matmuls + exp + masking, PV lagging one jb.
                Returns the pending (final) PV arguments."""
                jl = jlists[ib]
                nj = len(jl)

                def emit_pv(jb, P, runs, ji):
                    for h in range(HPC):
                        for ri, (k0, k1) in enumerate(runs):
                            nc.tensor.matmul(
                                Ops[h][:, k0 * 128:k1 * 128],
                                Vt[jb][:, h * VS:(h + 1) * VS],
                                P[:, h * SB + k0 * 128:h * SB + k1 * 128],
                                start=(ji == 0 and ri == 0),
                                stop=(ji == nj - 1 and ri == len(runs) - 1),
                                skip_group_check=True)

                pend = None
                for ji, (jb, runs, xs) in enumerate(jl):
                    P = pp.tile([JB, HPC * SB], BF16, tag="P", name="P")
                    P3 = P[:].rearrange("p (h i) -> p h i", h=HPC)
                    for g in range(2):  # head pairs share one score tile
                        St = psum.tile([JB, 2 * SB], F32, tag="S", bufs=2,
                                       name="St")
                        St3 = St[:].rearrange("p (z i) -> p z i", z=2)
                        for z in range(2):
                            bp = 64 * z
                            for k0, k1 in runs:
                                nc.tensor.matmul(
                                    St[:, z * SB + k0 * 128:
                                       z * SB + k1 * 128],
                                    KT[g][bp:bp + 64,
                                          jb * JB:(jb + 1) * JB],
                                    QT[g][bp:bp + 64,
                                          ib * SB + k0 * 128:
                                          ib * SB + k1 * 128],
                                    start=True, stop=True)
                        Pg = P[:, 2 * g * SB:(2 * g + 2) * SB].rearrange(
                            "p (z i) -> p z i", z=2)
                        if runs == [(0, SB // 128)]:
                            # full block: flat contiguous AP (cheaper than
                            # the strided 3D form on the ACT engine)
                            nc.scalar.activation(
                                P[:, 2 * g * SB:(2 * g + 2) * SB],
                                St[:],
                                mybir.ActivationFunctionType.Exp)
                        else:
                            for k0, k1 in runs:
                                nc.scalar.activation(
                                    Pg[:, :, k0 * 128:k1 * 128],
                                    St3[:, :, k0 * 128:k1 * 128],
                                    mybir.ActivationFunctionType.Exp)
                    for k, bi in xs:
                        nc.gpsimd.tensor_mul(
                            P3[:, :, k * 128:(k + 1) * 128],
                            P3[:, :, k * 128:(k + 1) * 128],
                            mb[bi][:].rearrange("p (h i) -> p h i", h=HPC))
                    # PV for block j-1 emitted after S/exp of block j so the
                    # PE never blocks on ACT (software pipeline)
                    if pend is not None:
                        emit_pv(*pend)
                    pend = (jb, P[:], runs, ji)
                return lambda: emit_pv(*pend)

            def norm(ib, Ops, last):
                # heads pairwise-share one partition-broadcast on GPSIMD
                for g in range(2):
                    ls = pnorm.tile([1, 2 * SB], F32, tag="ls", name="ls")
                    for z in range(2):
                        lsl = ls[:, z * SB:(z + 1) * SB]
                        if last:  # ACT is idle in the tail
                            nc.scalar.copy(lsl, Ops[2 * g + z][DK:VS, :])
                        else:
                            nc.vector.tensor_copy(lsl, Ops[2 * g + z][DK:VS, :])
                    rr = pnorm.tile([1, 2 * SB], F32, tag="rr", name="rr")
                    nc.vector.reciprocal_approx_fast(rr[:], ls[:])
                    Rc = pnorm.tile([DK, 2 * SB], F32, tag="rc", name="Rc")
                    nc.gpsimd.partition_broadcast(Rc[:], rr[:])
                    for z in range(2):
                        h = 2 * g + z
                        hp, bp = h // 2, 64 * (h % 2)
                        nc.vector.tensor_mul(
                            XT[hp][bp:bp + 64, ib * SB:(ib + 1) * SB],
                            Ops[h][0:DK, :], Rc[:, z * SB:(z + 1) * SB])

            def outproj(ib, last):
                for jtp in range(4):
                    ob = pob.tile([128, 2 * SB], BF16, tag="ob", name="ob")
                    for t2 in range(2):
                        jt = 2 * jtp + t2
                        po = psum.tile([128, SB], F32, tag="S", bufs=2,
                                       name="po")
                        for ot in range(2):
                            nc.tensor.matmul(
                                po[:],
                                wo3[:, ot, jt * 128:(jt + 1) * 128],
                                XT[ot][:, ib * SB:(ib + 1) * SB],
                                start=(ot == 0), stop=(ot == 1))
                        dob = ob[:, t2 * SB:(t2 + 1) * SB]
                        if last and t2 == 0:  # split tail evicts ACT/DVE
                            nc.scalar.copy(dob, po[:])
                        else:
                            nc.vector.tensor_copy(dob, po[:])
                    dst = outd[jtp * 256:(jtp + 1) * 256,
                               ib * SB:(ib + 1) * SB].rearrange(
                                   "(t p) i -> p t i", t=2)
                    # SWDGE ring: keeps the HWDGE (sync) ring free for the
                    # activation prefetches (no head-of-line blocking)
                    nc.gpsimd.dma_start(dst, ob[:].rearrange(
                        "p (t i) -> p t i", t=2))

            proj_qk(0, QT, wq3, "q", None if not has_b else bq_t)
            proj_qk(0, KT, wk3, "k", None if not has_b else bk_t)
            proj_v(0)
            for sb in range(NSB):
                Ops = [psum.tile([VS, SB], F32, tag="O", bufs=4, name="Ops")
                       for _ in range(HPC)]
                pv_flush = attention_scores(sb, Ops)
                last = sb + 1 >= NSB
                # Q-projection of the next block fills the PE while ACT
                # finishes the tail exps; then the final PV drains.
                if not last:
                    proj_qk(sb + 1, QT, wq3, "q",
                            None if not has_b else bq_t)
                pv_flush()
                norm(sb, Ops, last)
                if not last:
                    proj_qk(sb + 1, KT, wk3, "k",
                            None if not has_b else bk_t)
                    proj_v(sb + 1)
                if sb == 0:
                    fetch_acts("q", qTd, 1)
                    fetch_acts("k", kTd, 1)
                    fetch_acts("v", vTd, 1)
                outproj(sb, last)
    nc.finalize()
    return nc


def _pack_w(wT, nchunk, width):
    """[nchunk*128, width] -> [128, nchunk*width] (d-chunks side by side)."""
    return np.ascontiguousarray(
        wT.reshape(nchunk, 128, width).transpose(1, 0, 2).reshape(
            128, nchunk * width)).astype(BF)


def kernel(q, k, v, mask, wq, bq, wk, bk, wv, bv, wo, bo):
    global LAST_RUN
    q, k, v = (np.asarray(x, np.float32) for x in (q, k, v))
    wq, bq, wk, bk = (np.asarray(x, np.float32) for x in (wq, bq, wk, bk))
    wv, bv, wo, bo = (np.asarray(x, np.float32) for x in (wv, bv, wo, bo))
    mask2 = np.asarray(mask)[0, 0] != 0

    jlists, btiles = _classify_mask(mask2)
    nbias = len(btiles)
    maskb = (np.ascontiguousarray(
        np.concatenate([np.stack(btiles)] * HPC, axis=2)).astype(BF)
        if nbias else None)

    scale = np.float32(1.0 / np.sqrt(DK))
    bo_eff = (bo + wo @ bv).astype(np.float32)
    has_b = bool(np.any(bq) or np.any(bk))

    # per-batch transposed activations, shared across the 4 group-cores
    qTs = [np.ascontiguousarray(q[b].T).astype(BF) for b in range(B)]
    kTs = [np.ascontiguousarray(k[b].T).astype(BF) for b in range(B)]
    vTs = [np.ascontiguousarray(v[b].T).astype(BF) for b in range(B)]

    # per-group packed weight shards
    wqs, wks, wvs, wos, bqs, bks = [], [], [], [], [], []
    for g in range(GROUPS):
        rows = slice(g * OL, (g + 1) * OL)
        wqs.append(_pack_w((wq[rows] * scale).T, ND, OL))
        wks.append(_pack_w(wk[rows].T, ND, OL))
        wvs.append(_pack_w(wv[rows].T, ND, OL))
        wos.append(_pack_w(np.ascontiguousarray(wo[:, rows].T), 2, D))
        bqs.append(np.ascontiguousarray((bq[rows] * scale)[:, None]))
        bks.append(np.ascontiguousarray(bk[rows][:, None]))

    in_maps = []
    for c in range(NCORES):
        b, g = c // GROUPS, c % GROUPS
        m = {
            "qT": qTs[b], "kT": kTs[b], "vT": vTs[b],
            "wqp": wqs[g], "wkp": wks[g], "wvp": wvs[g], "wop": wos[g],
        }
        if has_b:
            m["bq"] = bqs[g]
            m["bk"] = bks[g]
        if nbias:
            m["maskb"] = maskb
        in_maps.append(m)

    nc = _build(jlists, nbias, has_b)
    res = run_bass_kernel_spmd(nc, in_maps, core_ids=list(range(NCORES)))
    LAST_RUN = res
    if res.exec_time_ns is not None:
        print(f"HW exec time: {res.exec_time_ns} ns")

    outp = np.zeros((B, S, D), np.float32)
    for c in range(NCORES):
        b = c // GROUPS
        outp[b] += res.results[c]["out"].astype(np.float32).T
    outp += bo_eff
    return outp


# revision 36
# speedup vs baseline: 1.2612x; 1.2612x over previous
"""Multi-head attention block (B=2, S=2048, D=1024, H=16) on 8 trn2 cores.

Sharding: core c = (batch b = c//4, head-group g = c%4); each core computes
4 heads of one batch (Megatron column-shard of wq/wk/wv, row-shard of wo,
combined with data-parallel over batch). Host sums the 4 partial outputs
per batch and adds the (folded) bias.

v3 (pipelined bf16):
  - all matmul operands bf16 (halves DMA + fast weight load); PSUM fp32.
  - streaming structure per sb-block with the Q-projection of the next
    block interleaved before the last PV so the PE never waits on ACT;
    Tile overlaps engines across stages, PE stays HAM-warm.
  - causal structure per (ib, jb): only the visible i-suffix is computed
    by the S/exp/PV chain (runs); diagonal blocks get one 0/1 mask
    multiply on DVE; no zero-fill of masked regions (PV only contracts
    visible columns; per-column PSUM init via the has_written bit).
  - softmax row-sums via a ones-column appended to V (VS=65); softmax
    skips max-subtraction (scores are O(1)); r=1/l on DVE,
    partition-broadcast on GPSIMD.
  - PSUM budget (8 banks): scores tag "S" 2 x [128,1024]f32 slots (2
    banks each) shared with projection/out-proj accumulators; tag "O"
    4 x 1 bank for the PV accumulators.
"""

import numpy as np
import ml_dtypes

import concourse.bass as bass
import concourse.mybir as mybir
import concourse.tile as tile
from concourse import bacc
from concourse.bass_utils import run_bass_kernel_spmd

B, S, D, H = 2, 2048, 1024, 16
DK = D // H                  # 64
NCORES = 8
GROUPS = NCORES // B         # 4 head-groups
HPC = H // GROUPS            # 4 heads per core
OL = HPC * DK                # 256 local features
SB = 512                     # query (i) block
JB = 128                     # key (j) block
NSB = S // SB                # 4
NJB = S // JB                # 16
VS = DK + 1                  # V cols per head incl. ones column (65)
ND = D // 128                # 8 contraction chunks

F32 = mybir.dt.float32
BF16 = mybir.dt.bfloat16
BF = ml_dtypes.bfloat16

LAST_RUN = None  # stash of BassKernelResults for test harness inspection


def _classify_mask(mask2):
    """Per (ib, jb) schedule from the boolean mask [S, S] (True = visible).

    Returns (jlists, bias_tiles):
      jlists[ib] = list of (jb, runs, xs) for j-blocks with any visible
        entry; runs = [(k0, k1)] contiguous non-fully-masked i-subblock
        ranges (128 wide); xs = [(k, bias_idx)] partially-masked
        subblocks needing a 0/1 multiply.
      bias_tiles: deduped [JB, 128] f32 0/1 tiles (transposed: [j, i]).
    """
    jlists = []
    btiles = []
    bkeys = {}
    assert mask2.any(axis=1).all(), "mask has a fully-masked query row"
    for ib in range(NSB):
        jl = []
        for jb in range(NJB):
            sub = mask2[ib * SB:(ib + 1) * SB, jb * JB:(jb + 1) * JB]
            if not sub.any():
                continue
            runs = []
            xs = []
            start = None
            for k in range(SB // 128):
                s2 = sub[k * 128:(k + 1) * 128, :]
                if not s2.any():
                    if start is not None:
                        runs.append((start, k))
                        start = None
                    continue
                if start is None:
                    start = k
                if not s2.all():
                    t = np.where(s2, np.float32(1), np.float32(0)).T
                    key = t.tobytes()
                    if key not in bkeys:
                        bkeys[key] = len(btiles)
                        btiles.append(t)
                    xs.append((k, bkeys[key]))
            if start is not None:
                runs.append((start, SB // 128))
            jl.append((jb, runs, xs))
        # widest blocks first, narrowest last: the final block's exp (which
        # the last PV and the normalization wait on) is as short as possible.
        # Order is free: each PSUM column is initialized by its first writer
        # via the has_written bit (cleared once by the start=True matmul).
        jl.sort(key=lambda e: -sum(k1 - k0 for k0, k1 in e[1]))
        jlists.append(jl)
    return jlists, btiles


def _build(jlists, nbias, has_b):
    nc = bacc.Bacc()

    qTd = nc.dram_tensor("qT", [D, S], BF16, kind="ExternalInput")
    kTd = nc.dram_tensor("kT", [D, S], BF16, kind="ExternalInput")
    vTd = nc.dram_tensor("vT", [D, S], BF16, kind="ExternalInput")
    wqd = nc.dram_tensor("wqp", [128, ND * OL], BF16, kind="ExternalInput")
    wkd = nc.dram_tensor("wkp", [128, ND * OL], BF16, kind="ExternalInput")
    wvd = nc.dram_tensor("wvp", [128, ND * OL], BF16, kind="ExternalInput")
    wod = nc.dram_tensor("wop", [128, 2 * D], BF16, kind="ExternalInput")
    if has_b:
        bqd = nc.dram_tensor("bq", [OL, 1], F32, kind="ExternalInput")
        bkd = nc.dram_tensor("bk", [OL, 1], F32, kind="ExternalInput")
    if nbias:
        mbd = nc.dram_tensor("maskb", [nbias, JB, HPC * 128], BF16,
                             kind="ExternalInput")
    outd = nc.dram_tensor("out", [D, S], BF16, kind="ExternalOutput")

    with tile.TileContext(nc) as tc:
        with tc.tile_pool(name="consts", bufs=1) as consts, \
             tc.tile_pool(name="acts", bufs=48) as actp, \
             tc.tile_pool(name="pp", bufs=3) as pp, \
             tc.tile_pool(name="pnorm", bufs=2) as pnorm, \
             tc.tile_pool(name="pob", bufs=3) as pob, \
             tc.tile_pool(name="ps", bufs=2, space="PSUM") as psum:

            QT = [consts.tile([128, S], BF16, name=f"QT{t}") for t in range(2)]
            KT = [consts.tile([128, S], BF16, name=f"KT{t}") for t in range(2)]
            XT = [consts.tile([128, S], BF16, name=f"XT{t}") for t in range(2)]
            Vt = [consts.tile([128, HPC * VS], BF16, name=f"V{st}")
                  for st in range(S // 128)]
            wq_t = consts.tile([128, ND * OL], BF16, name="wqt")
            wk_t = consts.tile([128, ND * OL], BF16, name="wkt")
            wv_t = consts.tile([128, ND * OL], BF16, name="wvt")
            wo_t = consts.tile([128, 2 * D], BF16, name="wot")
            mb = [consts.tile([JB, HPC * 128], BF16, name=f"mb{i}")
                  for i in range(nbias)]
            dummy = consts.tile([1, 8], BF16, name="dummy")
            if has_b:
                bq_t = [consts.tile([128, 1], F32, name=f"bq{t}")
                        for t in range(2)]
                bk_t = [consts.tile([128, 1], F32, name=f"bk{t}")
                        for t in range(2)]

            # preload the exp table set during the initial DMA stall
            nc.vector.memset(dummy[:], 0.0)
            nc.scalar.activation(dummy[:], dummy[:],
                                 mybir.ActivationFunctionType.Exp)
            # warm the PE HAM clock-gate during the initial DMA stall:
            # ~3.4us of sustained matmuls lifts the PE from 1.2 to 2.4 GHz
            # before the first projection matmul issues (dummy data)
            wrm = consts.tile([128, SB], BF16, name="wrm")
            nc.vector.memset(wrm[:], 0.0)
            psw = psum.tile([128, SB], F32, tag="S", bufs=2, name="psw")
            for _ in range(22):
                nc.tensor.matmul(psw[:], wrm[:, 0:128], wrm[:],
                                 start=True, stop=True,
                                 skip_group_check=True)
            # ones columns of V (memset once; V evictions write cols 0:DK)
            for st in range(S // 128):
                v3 = Vt[st][:].rearrange("p (h c) -> p h c", c=VS)
                nc.vector.memset(v3[:, :, DK:VS], 1.0)

            act_tiles = {}

            def fetch_acts(tname, dram, pair):
                for d in range(ND):
                    at = actp.tile([128, 2 * SB], BF16, tag="act", bufs=48,
                                   name="at")
                    nc.sync.dma_start(
                        at[:],
                        dram[d * 128:(d + 1) * 128,
                             pair * 2 * SB:(pair + 1) * 2 * SB])
                    act_tiles[(tname, d, pair)] = at

            # setup DMAs in first-consumption order
            nc.sync.dma_start(wq_t[:], wqd[:, :])
            fetch_acts("q", qTd, 0)
            nc.sync.dma_start(wk_t[:], wkd[:, :])
            fetch_acts("k", kTd, 0)
            nc.sync.dma_start(wv_t[:], wvd[:, :])
            fetch_acts("v", vTd, 0)
            for i in range(nbias):
                nc.sync.dma_start(mb[i][:], mbd[i])
            nc.sync.dma_start(wo_t[:], wod[:, :])
            if has_b:
                for t in range(2):
                    nc.sync.dma_start(bq_t[t][:],
                                      bqd[t * 128:(t + 1) * 128, :])
                    nc.sync.dma_start(bk_t[t][:],
                                      bkd[t * 128:(t + 1) * 128, :])

            wq3 = wq_t[:].rearrange("p (d c) -> p d c", d=ND)
            wk3 = wk_t[:].rearrange("p (d c) -> p d c", d=ND)
            wv3 = wv_t[:].rearrange("p (d c) -> p d c", d=ND)
            wo3 = wo_t[:].rearrange("p (t c) -> p t c", t=2)

            def proj_qk(sb, dst, wview, src, bias):
                # psum[feat 128, i 512] = sum_d w[d, feat].T @ act[d, i]
                pair, half = sb // 2, sb % 2
                cols = slice(half * SB, (half + 1) * SB)
                ps = [psum.tile([128, SB], F32, tag="S", bufs=2, name="pjq")
                      for _ in range(2)]
                for d in range(ND):
                    at = act_tiles[(src, d, pair)]
                    for ot in range(2):
                        nc.tensor.matmul(
                            ps[ot][:],
                            wview[:, d, ot * 128:(ot + 1) * 128],
                            at[:, cols],
                            start=(d == 0), stop=(d == ND - 1))
                for ot in range(2):
                    dcols = dst[ot][:, sb * SB:(sb + 1) * SB]
                    if has_b:
                        nc.scalar.add(dcols, ps[ot][:], bias[ot][:])
                    else:
                        nc.scalar.copy(dcols, ps[ot][:])

            def proj_v(sb):
                # psum[s-subblock 128, feat 256] = act[d, s].T @ wv[d, :]
                pair, half = sb // 2, sb % 2
                for halfk in range(2):
                    psv = psum.tile([128, 2 * OL], F32, tag="S", bufs=2,
                                    name="pjv")
                    for k2 in range(2):
                        k = 2 * halfk + k2
                        for d in range(ND):
                            at = act_tiles[("v", d, pair)]
                            nc.tensor.matmul(
                                psv[:, k2 * OL:(k2 + 1) * OL],
                                at[:, half * SB + k * 128:
                                   half * SB + (k + 1) * 128],
                                wv3[:, d, :],
                                start=(d == 0), stop=(d == ND - 1),
                                skip_group_check=True)
                    for k2 in range(2):
                        st = sb * 4 + 2 * halfk + k2
                        v3 = Vt[st][:].rearrange("p (h c) -> p h c", c=VS)
                        nc.vector.tensor_copy(
                            v3[:, :, 0:DK],
                            psv[:, k2 * OL:(k2 + 1) * OL].rearrange(
                                "p (h c) -> p h c", c=DK))

            def attention_scores(ib, Ops):
                """jb loop: S matmuls + exp + masking, PV lagging one jb.
                Returns the pending (final) PV arguments."""
                jl = jlists[ib]
                nj = len(jl)

                def emit_pv(jb, P, runs, ji):
                    for h in range(HPC):
                        for ri, (k0, k1) in enumerate(runs):
                            nc.tensor.matmul(
                                Ops[h][:, k0 * 128:k1 * 128],
                                Vt[jb][:, h * VS:(h + 1) * VS],
                                P[:, h * SB + k0 * 128:h * SB + k1 * 128],
                                start=(ji == 0 and ri == 0),
                                stop=(ji == nj - 1 and ri == len(runs) - 1),
                                skip_group_check=True)

                pend = None
                for ji, (jb, runs, xs) in enumerate(jl):
                    P = pp.tile([JB, HPC * SB], BF16, tag="P", name="P")
                    P3 = P[:].rearrange("p (h i) -> p h i", h=HPC)
                    for g in range(2):  # head pairs share one score tile
                        St = psum.tile([JB, 2 * SB], F32, tag="S", bufs=2,
                                       name="St")
                        St3 = St[:].rearrange("p (z i) -> p z i", z=2)
                        for z in range(2):
                            bp = 64 * z
                            for k0, k1 in runs:
                                nc.tensor.matmul(
                                    St[:, z * SB + k0 * 128:
                                       z * SB + k1 * 128],
                                    KT[g][bp:bp + 64,
                                          jb * JB:(jb + 1) * JB],
                                    QT[g][bp:bp + 64,
                                          ib * SB + k0 * 128:
                                          ib * SB + k1 * 128],
                                    start=True, stop=True)
                        Pg = P[:, 2 * g * SB:(2 * g + 2) * SB].rearrange(
                            "p (z i) -> p z i", z=2)
                        if runs == [(0, SB // 128)]:
                            # full block: flat contiguous AP (cheaper than
                            # the strided 3D form on the ACT engine)
                            nc.scalar.activation(
                                P[:, 2 * g * SB:(2 * g + 2) * SB],
                                St[:],
                                mybir.ActivationFunctionType.Exp)
                        else:
                            for k0, k1 in runs:
                                nc.scalar.activation(
                                    Pg[:, :, k0 * 128:k1 * 128],
                                    St3[:, :, k0 * 128:k1 * 128],
                                    mybir.ActivationFunctionType.Exp)
                    for k, bi in xs:
                        nc.vector.tensor_mul(
                            P3[:, :, k * 128:(k + 1) * 128],
                            P3[:, :, k * 128:(k + 1) * 128],
                            mb[bi][:].rearrange("p (h i) -> p h i", h=HPC))
                    # PV for block j-1 emitted after S/exp of block j so the
                    # PE never blocks on ACT (software pipeline)
                    if pend is not None:
                        emit_pv(*pend)
                    pend = (jb, P[:], runs, ji)
                return lambda: emit_pv(*pend)

            def norm(ib, Ops, last):
                # heads pairwise-share one partition-broadcast on GPSIMD
                for g in range(2):
                    ls = pnorm.tile([1, 2 * SB], F32, tag="ls", name="ls")
                    for z in range(2):
                        lsl = ls[:, z * SB:(z + 1) * SB]
                        if last:  # ACT is idle in the tail
                            nc.scalar.copy(lsl, Ops[2 * g + z][DK:VS, :])
                        else:
                            nc.vector.tensor_copy(lsl, Ops[2 * g + z][DK:VS, :])
                    rr = pnorm.tile([1, 2 * SB], F32, tag="rr", name="rr")
                    nc.vector.reciprocal_approx_fast(rr[:], ls[:])
                    Rc = pnorm.tile([DK, 2 * SB], F32, tag="rc", name="Rc")
                    nc.gpsimd.partition_broadcast(Rc[:], rr[:])
                    for z in range(2):
                        h = 2 * g + z
                        hp, bp = h // 2, 64 * (h % 2)
                        nc.vector.tensor_mul(
                            XT[hp][bp:bp + 64, ib * SB:(ib + 1) * SB],
                            Ops[h][0:DK, :], Rc[:, z * SB:(z + 1) * SB])

            def outproj(ib, last):
                for jtp in range(4):
                    ob = pob.tile([128, 2 * SB], BF16, tag="ob", name="ob")
                    for t2 in range(2):
                        jt = 2 * jtp + t2
                        po = psum.tile([128, SB], F32, tag="S", bufs=2,
                                       name="po")
                        for ot in range(2):
                            nc.tensor.matmul(
                                po[:],
                                wo3[:, ot, jt * 128:(jt + 1) * 128],
                                XT[ot][:, ib * SB:(ib + 1) * SB],
                                start=(ot == 0), stop=(ot == 1))
                        dob = ob[:, t2 * SB:(t2 + 1) * SB]
                        if last and t2 == 0:  # split tail evicts ACT/DVE
                            nc.scalar.copy(dob, po[:])
                        else:
                            nc.vector.tensor_copy(dob, po[:])
                    dst = outd[jtp * 256:(jtp + 1) * 256,
                               ib * SB:(ib + 1) * SB].rearrange(
                                   "(t p) i -> p t i", t=2)
                    # SWDGE ring: keeps the HWDGE (sync) ring free for the
                    # activation prefetches (no head-of-line blocking)
                    nc.gpsimd.dma_start(dst, ob[:].rearrange(
                        "p (t i) -> p t i", t=2))

            proj_qk(0, QT, wq3, "q", None if not has_b else bq_t)
            proj_qk(0, KT, wk3, "k", None if not has_b else bk_t)
            proj_v(0)
            for sb in range(NSB):
                Ops = [psum.tile([VS, SB], F32, tag="O", bufs=4, name="Ops")
                       for _ in range(HPC)]
                pv_flush = attention_scores(sb, Ops)
                last = sb + 1 >= NSB
                # Q-projection of the next block fills the PE while ACT
                # finishes the tail exps; then the final PV drains.
                if not last:
                    proj_qk(sb + 1, QT, wq3, "q",
                            None if not has_b else bq_t)
                pv_flush()
                norm(sb, Ops, last)
                if not last:
                    proj_qk(sb + 1, KT, wk3, "k",
                            None if not has_b else bk_t)
                    proj_v(sb + 1)
                if sb == 0:
                    fetch_acts("q", qTd, 1)
                    fetch_acts("k", kTd, 1)
                    fetch_acts("v", vTd, 1)
                outproj(sb, last)
    nc.finalize()
    return nc


def _pack_w(wT, nchunk, width):
    """[nchunk*128, width] -> [128, nchunk*width] (d-chunks side by side)."""
    return np.ascontiguousarray(
        wT.reshape(nchunk, 128, width).transpose(1, 0, 2).reshape(
            128, nchunk * width)).astype(BF)


def kernel(q, k, v, mask, wq, bq, wk, bk, wv, bv, wo, bo):
    global LAST_RUN
    q, k, v = (np.asarray(x, np.float32) for x in (q, k, v))
    wq, bq, wk, bk = (np.asarray(x, np.float32) for x in (wq, bq, wk, bk))
    wv, bv, wo, bo = (np.asarray(x, np.float32) for x in (wv, bv, wo, bo))
    mask2 = np.asarray(mask)[0, 0] != 0

    jlists, btiles = _classify_mask(mask2)
    nbias = len(btiles)
    maskb = (np.ascontiguousarray(
        np.concatenate([np.stack(btiles)] * HPC, axis=2)).astype(BF)
        if nbias else None)

    scale = np.float32(1.0 / np.sqrt(DK))
    bo_eff = (bo + wo @ bv).astype(np.float32)
    has_b = bool(np.any(bq) or np.any(bk))

    # per-batch transposed activations, shared across the 4 group-cores
    qTs = [np.ascontiguousarray(q[b].T).astype(BF) for b in range(B)]
    kTs = [np.ascontiguousarray(k[b].T).astype(BF) for b in range(B)]
    vTs = [np.ascontiguousarray(v[b].T).astype(BF) for b in range(B)]

    # per-group packed weight shards
    wqs, wks, wvs, wos, bqs, bks = [], [], [], [], [], []
    for g in range(GROUPS):
        rows = slice(g * OL, (g + 1) * OL)
        wqs.append(_pack_w((wq[rows] * scale).T, ND, OL))
        wks.append(_pack_w(wk[rows].T, ND, OL))
        wvs.append(_pack_w(wv[rows].T, ND, OL))
        wos.append(_pack_w(np.ascontiguousarray(wo[:, rows].T), 2, D))
        bqs.append(np.ascontiguousarray((bq[rows] * scale)[:, None]))
        bks.append(np.ascontiguousarray(bk[rows][:, None]))

    in_maps = []
    for c in range(NCORES):
        b, g = c // GROUPS, c % GROUPS
        m = {
            "qT": qTs[b], "kT": kTs[b], "vT": vTs[b],
            "wqp": wqs[g], "wkp": wks[g], "wvp": wvs[g], "wop": wos[g],
        }
        if has_b:
            m["bq"] = bqs[g]
            m["bk"] = bks[g]
        if nbias:
            m["maskb"] = maskb
        in_maps.append(m)

    nc = _build(jlists, nbias, has_b)
    res = run_bass_kernel_spmd(nc, in_maps, core_ids=list(range(NCORES)))
    LAST_RUN = res
    if res.exec_time_ns is not None:
        print(f"HW exec time: {res.exec_time_ns} ns")

    outp = np.zeros((B, S, D), np.float32)
    for c in range(NCORES):
        b = c // GROUPS
        outp[b] += res.results[c]["out"].astype(np.float32).T
    outp += bo_eff
    return outp


# revision 37
# speedup vs baseline: 1.2740x; 1.0101x over previous
"""Multi-head attention block (B=2, S=2048, D=1024, H=16) on 8 trn2 cores.

Sharding: core c = (batch b = c//4, head-group g = c%4); each core computes
4 heads of one batch (Megatron column-shard of wq/wk/wv, row-shard of wo,
combined with data-parallel over batch). Host sums the 4 partial outputs
per batch and adds the (folded) bias.

Implementation (pipelined bf16):
  - all matmul operands bf16 (halves DMA + fast weight load); PSUM fp32.
  - streaming structure per sb-block with the Q-projection of the next
    block interleaved before the last PV so the PE never waits on ACT;
    Tile overlaps engines across stages, PE stays HAM-warm.
  - causal structure per (ib, jb): only the visible i-suffix is computed
    by the S/exp/PV chain (runs); diagonal blocks get one 0/1 mask
    multiply on DVE; no zero-fill of masked regions (PV only contracts
    visible columns; per-column PSUM init via the has_written bit).
  - softmax row-sums via a ones-column appended to V (VS=65); softmax
    skips max-subtraction (scores are O(1)); r=1/l on DVE,
    partition-broadcast on GPSIMD.
  - PSUM budget (8 banks): scores tag "S" 2 x [128,1024]f32 slots (2
    banks each) shared with projection/out-proj accumulators; tag "O"
    4 x 1 bank for the PV accumulators.
"""

import numpy as np
import ml_dtypes

import concourse.bass as bass
import concourse.mybir as mybir
import concourse.tile as tile
from concourse import bacc
from concourse.bass_utils import run_bass_kernel_spmd

B, S, D, H = 2, 2048, 1024, 16
DK = D // H                  # 64
NCORES = 8
GROUPS = NCORES // B         # 4 head-groups
HPC = H // GROUPS            # 4 heads per core
OL = HPC * DK                # 256 local features
SB = 512                     # query (i) block
JB = 128                     # key (j) block
NSB = S // SB                # 4
NJB = S // JB                # 16
VS = DK + 1                  # V cols per head incl. ones column (65)
ND = D // 128                # 8 contraction chunks

F32 = mybir.dt.float32
BF16 = mybir.dt.bfloat16
BF = ml_dtypes.bfloat16

LAST_RUN = None  # stash of BassKernelResults for test harness inspection


def _classify_mask(mask2):
    """Per (ib, jb) schedule from the boolean mask [S, S] (True = visible).

    Returns (jlists, bias_tiles):
      jlists[ib] = list of (jb, runs, xs) for j-blocks with any visible
        entry; runs = [(k0, k1)] contiguous non-fully-masked i-subblock
        ranges (128 wide); xs = [(k, bias_idx)] partially-masked
        subblocks needing a 0/1 multiply.
      bias_tiles: deduped [JB, 128] f32 0/1 tiles (transposed: [j, i]).
    """
    jlists = []
    btiles = []
    bkeys = {}
    assert mask2.any(axis=1).all(), "mask has a fully-masked query row"
    for ib in range(NSB):
        jl = []
        for jb in range(NJB):
            sub = mask2[ib * SB:(ib + 1) * SB, jb * JB:(jb + 1) * JB]
            if not sub.any():
                continue
            runs = []
            xs = []
            start = None
            for k in range(SB // 128):
                s2 = sub[k * 128:(k + 1) * 128, :]
                if not s2.any():
                    if start is not None:
                        runs.append((start, k))
                        start = None
                    continue
                if start is None:
                    start = k
                if not s2.all():
                    t = np.where(s2, np.float32(1), np.float32(0)).T
                    key = t.tobytes()
                    if key not in bkeys:
                        bkeys[key] = len(btiles)
                        btiles.append(t)
                    xs.append((k, bkeys[key]))
            if start is not None:
                runs.append((start, SB // 128))
            jl.append((jb, runs, xs))
        # widest blocks first, narrowest last: the final block's exp (which
        # the last PV and the normalization wait on) is as short as possible.
        # Order is free: each PSUM column is initialized by its first writer
        # via the has_written bit (cleared once by the start=True matmul).
        jl.sort(key=lambda e: -sum(k1 - k0 for k0, k1 in e[1]))
        jlists.append(jl)
    return jlists, btiles


def _build(jlists, nbias, has_b):
    nc = bacc.Bacc()

    qTd = nc.dram_tensor("qT", [D, S], BF16, kind="ExternalInput")
    kTd = nc.dram_tensor("kT", [D, S], BF16, kind="ExternalInput")
    vTd = nc.dram_tensor("vT", [D, S], BF16, kind="ExternalInput")
    wqd = nc.dram_tensor("wqp", [128, ND * OL], BF16, kind="ExternalInput")
    wkd = nc.dram_tensor("wkp", [128, ND * OL], BF16, kind="ExternalInput")
    wvd = nc.dram_tensor("wvp", [128, ND * OL], BF16, kind="ExternalInput")
    wod = nc.dram_tensor("wop", [128, 2 * D], BF16, kind="ExternalInput")
    if has_b:
        bqd = nc.dram_tensor("bq", [OL, 1], F32, kind="ExternalInput")
        bkd = nc.dram_tensor("bk", [OL, 1], F32, kind="ExternalInput")
    if nbias:
        mbd = nc.dram_tensor("maskb", [nbias, JB, HPC * 128], BF16,
                             kind="ExternalInput")
    outd = nc.dram_tensor("out", [D, S], BF16, kind="ExternalOutput")

    with tile.TileContext(nc) as tc:
        with tc.tile_pool(name="consts", bufs=1) as consts, \
             tc.tile_pool(name="acts", bufs=48) as actp, \
             tc.tile_pool(name="pp", bufs=3) as pp, \
             tc.tile_pool(name="pnorm", bufs=2) as pnorm, \
             tc.tile_pool(name="pob", bufs=3) as pob, \
             tc.tile_pool(name="ps", bufs=2, space="PSUM") as psum:

            QT = [consts.tile([128, S], BF16, name=f"QT{t}") for t in range(2)]
            KT = [consts.tile([128, S], BF16, name=f"KT{t}") for t in range(2)]
            XT = [consts.tile([128, S], BF16, name=f"XT{t}") for t in range(2)]
            Vt = [consts.tile([128, HPC * VS], BF16, name=f"V{st}")
                  for st in range(S // 128)]
            wq_t = consts.tile([128, ND * OL], BF16, name="wqt")
            wk_t = consts.tile([128, ND * OL], BF16, name="wkt")
            wv_t = consts.tile([128, ND * OL], BF16, name="wvt")
            wo_t = consts.tile([128, 2 * D], BF16, name="wot")
            mb = [consts.tile([JB, HPC * 128], BF16, name=f"mb{i}")
                  for i in range(nbias)]
            dummy = consts.tile([1, 8], BF16, name="dummy")
            if has_b:
                bq_t = [consts.tile([128, 1], F32, name=f"bq{t}")
                        for t in range(2)]
                bk_t = [consts.tile([128, 1], F32, name=f"bk{t}")
                        for t in range(2)]

            # preload the exp table set during the initial DMA stall
            nc.vector.memset(dummy[:], 0.0)
            nc.scalar.activation(dummy[:], dummy[:],
                                 mybir.ActivationFunctionType.Exp)
            # warm the PE HAM clock-gate during the initial DMA stall:
            # ~3.4us of sustained matmuls lifts the PE from 1.2 to 2.4 GHz
            # before the first projection matmul issues (dummy data)
            wrm = consts.tile([128, SB], BF16, name="wrm")
            nc.vector.memset(wrm[:], 0.0)
            psw = psum.tile([128, SB], F32, tag="S", bufs=2, name="psw")
            for _ in range(22):
                nc.tensor.matmul(psw[:], wrm[:, 0:128], wrm[:],
                                 start=True, stop=True,
                                 skip_group_check=True)
            # ones columns of V (memset once; V evictions write cols 0:DK)
            for st in range(S // 128):
                v3 = Vt[st][:].rearrange("p (h c) -> p h c", c=VS)
                nc.vector.memset(v3[:, :, DK:VS], 1.0)

            act_tiles = {}

            def fetch_acts(tname, dram, pair):
                for d in range(ND):
                    at = actp.tile([128, 2 * SB], BF16, tag="act", bufs=48,
                                   name="at")
                    nc.sync.dma_start(
                        at[:],
                        dram[d * 128:(d + 1) * 128,
                             pair * 2 * SB:(pair + 1) * 2 * SB])
                    act_tiles[(tname, d, pair)] = at

            # setup DMAs in first-consumption order
            nc.sync.dma_start(wq_t[:], wqd[:, :])
            fetch_acts("q", qTd, 0)
            nc.sync.dma_start(wk_t[:], wkd[:, :])
            fetch_acts("k", kTd, 0)
            nc.sync.dma_start(wv_t[:], wvd[:, :])
            fetch_acts("v", vTd, 0)
            for i in range(nbias):
                nc.sync.dma_start(mb[i][:], mbd[i])
            nc.sync.dma_start(wo_t[:], wod[:, :])
            if has_b:
                for t in range(2):
                    nc.sync.dma_start(bq_t[t][:],
                                      bqd[t * 128:(t + 1) * 128, :])
                    nc.sync.dma_start(bk_t[t][:],
                                      bkd[t * 128:(t + 1) * 128, :])

            wq3 = wq_t[:].rearrange("p (d c) -> p d c", d=ND)
            wk3 = wk_t[:].rearrange("p (d c) -> p d c", d=ND)
            wv3 = wv_t[:].rearrange("p (d c) -> p d c", d=ND)
            wo3 = wo_t[:].rearrange("p (t c) -> p t c", t=2)

            def proj_qk(sb, dst, wview, src, bias):
                # psum[feat 128, i 512] = sum_d w[d, feat].T @ act[d, i]
                pair, half = sb // 2, sb % 2
                cols = slice(half * SB, (half + 1) * SB)
                ps = [psum.tile([128, SB], F32, tag="S", bufs=2, name="pjq")
                      for _ in range(2)]
                for d in range(ND):
                    at = act_tiles[(src, d, pair)]
                    for ot in range(2):
                        nc.tensor.matmul(
                            ps[ot][:],
                            wview[:, d, ot * 128:(ot + 1) * 128],
                            at[:, cols],
                            start=(d == 0), stop=(d == ND - 1))
                for ot in range(2):
                    dcols = dst[ot][:, sb * SB:(sb + 1) * SB]
                    if has_b:
                        nc.scalar.add(dcols, ps[ot][:], bias[ot][:])
                    else:
                        nc.scalar.copy(dcols, ps[ot][:])

            def proj_v(sb):
                # psum[s-subblock 128, feat 256] = act[d, s].T @ wv[d, :]
                pair, half = sb // 2, sb % 2
                for halfk in range(2):
                    psv = psum.tile([128, 2 * OL], F32, tag="S", bufs=2,
                                    name="pjv")
                    for k2 in range(2):
                        k = 2 * halfk + k2
                        for d in range(ND):
                            at = act_tiles[("v", d, pair)]
                            nc.tensor.matmul(
                                psv[:, k2 * OL:(k2 + 1) * OL],
                                at[:, half * SB + k * 128:
                                   half * SB + (k + 1) * 128],
                                wv3[:, d, :],
                                start=(d == 0), stop=(d == ND - 1),
                                skip_group_check=True)
                    for k2 in range(2):
                        st = sb * 4 + 2 * halfk + k2
                        v3 = Vt[st][:].rearrange("p (h c) -> p h c", c=VS)
                        nc.vector.tensor_copy(
                            v3[:, :, 0:DK],
                            psv[:, k2 * OL:(k2 + 1) * OL].rearrange(
                                "p (h c) -> p h c", c=DK))

            def attention_scores(ib, Ops):
                """jb loop: S matmuls + exp + masking, PV lagging one jb.
                Returns the pending (final) PV arguments."""
                jl = jlists[ib]
                nj = len(jl)

                def emit_pv(jb, P, runs, ji):
                    for h in range(HPC):
                        for ri, (k0, k1) in enumerate(runs):
                            nc.tensor.matmul(
                                Ops[h][:, k0 * 128:k1 * 128],
                                Vt[jb][:, h * VS:(h + 1) * VS],
                                P[:, h * SB + k0 * 128:h * SB + k1 * 128],
                                start=(ji == 0 and ri == 0),
                                stop=(ji == nj - 1 and ri == len(runs) - 1),
                                skip_group_check=True)

                pend = None
                for ji, (jb, runs, xs) in enumerate(jl):
                    P = pp.tile([JB, HPC * SB], BF16, tag="P", name="P")
                    P3 = P[:].rearrange("p (h i) -> p h i", h=HPC)
                    for g in range(2):  # head pairs share one score tile
                        St = psum.tile([JB, 2 * SB], F32, tag="S", bufs=2,
                                       name="St")
                        St3 = St[:].rearrange("p (z i) -> p z i", z=2)
                        for z in range(2):
                            bp = 64 * z
                            for k0, k1 in runs:
                                nc.tensor.matmul(
                                    St[:, z * SB + k0 * 128:
                                       z * SB + k1 * 128],
                                    KT[g][bp:bp + 64,
                                          jb * JB:(jb + 1) * JB],
                                    QT[g][bp:bp + 64,
                                          ib * SB + k0 * 128:
                                          ib * SB + k1 * 128],
                                    start=True, stop=True)
                        Pg = P[:, 2 * g * SB:(2 * g + 2) * SB].rearrange(
                            "p (z i) -> p z i", z=2)
                        if runs == [(0, SB // 128)]:
                            # full block: flat contiguous AP (cheaper than
                            # the strided 3D form on the ACT engine)
                            nc.scalar.activation(
                                P[:, 2 * g * SB:(2 * g + 2) * SB],
                                St[:],
                                mybir.ActivationFunctionType.Exp)
                        else:
                            for k0, k1 in runs:
                                nc.scalar.activation(
                                    Pg[:, :, k0 * 128:k1 * 128],
                                    St3[:, :, k0 * 128:k1 * 128],
                                    mybir.ActivationFunctionType.Exp)
                    for k, bi in xs:
                        nc.vector.tensor_mul(
                            P3[:, :, k * 128:(k + 1) * 128],
                            P3[:, :, k * 128:(k + 1) * 128],
                            mb[bi][:].rearrange("p (h i) -> p h i", h=HPC))
                    # PV for block j-1 emitted after S/exp of block j so the
                    # PE never blocks on ACT (software pipeline)
                    if pend is not None:
                        emit_pv(*pend)
                    pend = (jb, P[:], runs, ji)
                return lambda: emit_pv(*pend)

            def norm(ib, Ops, last):
                # heads pairwise-share one partition-broadcast on GPSIMD
                for g in range(2):
                    ls = pnorm.tile([1, 2 * SB], F32, tag="ls", name="ls")
                    for z in range(2):
                        lsl = ls[:, z * SB:(z + 1) * SB]
                        if last:  # ACT is idle in the tail
                            nc.scalar.copy(lsl, Ops[2 * g + z][DK:VS, :])
                        else:
                            nc.vector.tensor_copy(lsl, Ops[2 * g + z][DK:VS, :])
                    rr = pnorm.tile([1, 2 * SB], F32, tag="rr", name="rr")
                    nc.vector.reciprocal_approx_fast(rr[:], ls[:])
                    Rc = pnorm.tile([DK, 2 * SB], F32, tag="rc", name="Rc")
                    nc.gpsimd.partition_broadcast(Rc[:], rr[:])
                    for z in range(2):
                        h = 2 * g + z
                        hp, bp = h // 2, 64 * (h % 2)
                        nc.vector.tensor_mul(
                            XT[hp][bp:bp + 64, ib * SB:(ib + 1) * SB],
                            Ops[h][0:DK, :], Rc[:, z * SB:(z + 1) * SB])

            def outproj(ib, last):
                for jtp in range(4):
                    ob = pob.tile([128, 2 * SB], BF16, tag="ob", name="ob")
                    for t2 in range(2):
                        jt = 2 * jtp + t2
                        po = psum.tile([128, SB], F32, tag="S", bufs=2,
                                       name="po")
                        for ot in range(2):
                            nc.tensor.matmul(
                                po[:],
                                wo3[:, ot, jt * 128:(jt + 1) * 128],
                                XT[ot][:, ib * SB:(ib + 1) * SB],
                                start=(ot == 0), stop=(ot == 1))
                        dob = ob[:, t2 * SB:(t2 + 1) * SB]
                        if last and t2 == 0:  # split tail evicts ACT/DVE
                            nc.scalar.copy(dob, po[:])
                        else:
                            nc.vector.tensor_copy(dob, po[:])
                    dst = outd[jtp * 256:(jtp + 1) * 256,
                               ib * SB:(ib + 1) * SB].rearrange(
                                   "(t p) i -> p t i", t=2)
                    # SWDGE ring: keeps the HWDGE (sync) ring free for the
                    # activation prefetches (no head-of-line blocking)
                    nc.gpsimd.dma_start(dst, ob[:].rearrange(
                        "p (t i) -> p t i", t=2))

            proj_qk(0, QT, wq3, "q", None if not has_b else bq_t)
            proj_qk(0, KT, wk3, "k", None if not has_b else bk_t)
            proj_v(0)
            for sb in range(NSB):
                Ops = [psum.tile([VS, SB], F32, tag="O", bufs=4, name="Ops")
                       for _ in range(HPC)]
                pv_flush = attention_scores(sb, Ops)
                last = sb + 1 >= NSB
                # Q-projection of the next block fills the PE while ACT
                # finishes the tail exps; then the final PV drains.
                if not last:
                    proj_qk(sb + 1, QT, wq3, "q",
                            None if not has_b else bq_t)
                pv_flush()
                norm(sb, Ops, last)
                if not last:
                    proj_qk(sb + 1, KT, wk3, "k",
                            None if not has_b else bk_t)
                    proj_v(sb + 1)
                if sb == 0:
                    fetch_acts("q", qTd, 1)
                    fetch_acts("k", kTd, 1)
                    fetch_acts("v", vTd, 1)
                outproj(sb, last)
    nc.finalize()
    return nc


def _pack_w(wT, nchunk, width):
    """[nchunk*128, width] -> [128, nchunk*width] (d-chunks side by side)."""
    return np.ascontiguousarray(
        wT.reshape(nchunk, 128, width).transpose(1, 0, 2).reshape(
            128, nchunk * width)).astype(BF)


def kernel(q, k, v, mask, wq, bq, wk, bk, wv, bv, wo, bo):
    global LAST_RUN
    q, k, v = (np.asarray(x, np.float32) for x in (q, k, v))
    wq, bq, wk, bk = (np.asarray(x, np.float32) for x in (wq, bq, wk, bk))
    wv, bv, wo, bo = (np.asarray(x, np.float32) for x in (wv, bv, wo, bo))
    mask2 = np.asarray(mask)[0, 0] != 0

    jlists, btiles = _classify_mask(mask2)
    nbias = len(btiles)
    maskb = (np.ascontiguousarray(
        np.concatenate([np.stack(btiles)] * HPC, axis=2)).astype(BF)
        if nbias else None)

    scale = np.float32(1.0 / np.sqrt(DK))
    bo_eff = (bo + wo @ bv).astype(np.float32)
    has_b = bool(np.any(bq) or np.any(bk))

    # per-batch transposed activations, shared across the 4 group-cores
    qTs = [np.ascontiguousarray(q[b].T).astype(BF) for b in range(B)]
    kTs = [np.ascontiguousarray(k[b].T).astype(BF) for b in range(B)]
    vTs = [np.ascontiguousarray(v[b].T).astype(BF) for b in range(B)]

    # per-group packed weight shards
    wqs, wks, wvs, wos, bqs, bks = [], [], [], [], [], []
    for g in range(GROUPS):
        rows = slice(g * OL, (g + 1) * OL)
        wqs.append(_pack_w((wq[rows] * scale).T, ND, OL))
        wks.append(_pack_w(wk[rows].T, ND, OL))
        wvs.append(_pack_w(wv[rows].T, ND, OL))
        wos.append(_pack_w(np.ascontiguousarray(wo[:, rows].T), 2, D))
        bqs.append(np.ascontiguousarray((bq[rows] * scale)[:, None]))
        bks.append(np.ascontiguousarray(bk[rows][:, None]))

    in_maps = []
    for c in range(NCORES):
        b, g = c // GROUPS, c % GROUPS
        m = {
            "qT": qTs[b], "kT": kTs[b], "vT": vTs[b],
            "wqp": wqs[g], "wkp": wks[g], "wvp": wvs[g], "wop": wos[g],
        }
        if has_b:
            m["bq"] = bqs[g]
            m["bk"] = bks[g]
        if nbias:
            m["maskb"] = maskb
        in_maps.append(m)

    nc = _build(jlists, nbias, has_b)
    res = run_bass_kernel_spmd(nc, in_maps, core_ids=list(range(NCORES)))
    LAST_RUN = res
    if res.exec_time_ns is not None:
        print(f"HW exec time: {res.exec_time_ns} ns")

    outp = np.zeros((B, S, D), np.float32)
    for c in range(NCORES):
        b = c // GROUPS
        outp[b] += res.results[c]["out"].astype(np.float32).T
    outp += bo_eff
    return outp
